# revision 7
# baseline (speedup 1.0000x reference)
"""Non-local block (NLB) Trainium2 kernel.

Data-parallel over batch: 8 samples -> 8 NeuronCores, one sample per core.
Per core (C=128 channels, n=4096 pixels, inter=64):

  scores_T[m, n] = x_m^T (B x_n + v)        B = phi_w^T theta_w, v = phi_w^T theta_b
      (the per-n constant term theta_b . (phi_w x_n + phi_b) is dropped --
       softmax over m is invariant to it)
  probs_T = exp(scores_T)                   (scores bounded ~ +-22, fp32-safe,
                                             so no max-subtraction pass)
  y_aug[o, n]  = sum_m g_aug[m, o] probs_T[m, n]   with g_aug[:, 64] == 1
      -> row 64 is the softmax row-sum; g_aug[:, 0:64] = x^T g_w^T
  out[c, n] = (out_w @ y_aug[0:64]) [c, n] / y_aug[64, n] + out_b_eff[c] + x[c, n]
      out_b_eff = out_w @ g_b + out_b       (softmax rows sum to 1 => g_b folds)

Layouts keep the softmax (m) axis on partitions so exp runs straight out of
PSUM on ScalarE while the PE does scores / y matmuls in fp32r.
"""

import sys

if "/root/.axon_site/_ro/trn_rl_repo" not in sys.path:
    sys.path.insert(0, "/root/.axon_site/_ro/trn_rl_repo")

import types

import numpy as np

import concourse.bass as bass
import concourse.mybir as mybir
import concourse.tile as tile
from concourse import bacc
from concourse import bass_utils

# The image's antenv package lacks axon_hooks; shim it so trace=True works.
try:
    import antenv.axon_hooks  # noqa: F401
except ImportError:
    try:
        import trn_agent_boot.trn_boot as _tb

        _hook = _tb._ntff_profile_via_ctypes("/opt/axon/libaxon_pjrt.so")
        _m = types.ModuleType("antenv.axon_hooks")
        _m.get_axon_ntff_profile_hook = lambda: _hook
        sys.modules["antenv.axon_hooks"] = _m
    except Exception:
        pass

B, C, H, W = 8, 128, 64, 64
N = H * W          # 4096 pixels
INTER = C // 2     # 64
P = 128
NCH = 1024         # n-chunk width (exp batching; 2 PSUM banks)
NSUB = 512         # matmul moving-dim width (1 PSUM bank)
MBLK = N // P      # 32 m-blocks
F32 = mybir.dt.float32
F32R = mybir.dt.float32r

_cached = {}


def _r(ap):
    """View an fp32 AP as float32r for full-rate PE streaming."""
    return ap.bitcast(F32R)


def build_nc():
    nc = bacc.Bacc("TRN2", target_bir_lowering=False, debug=False, num_devices=B)

    x_d = nc.dram_tensor("x", [P, N], F32, kind="ExternalInput")
    ulhs_d = nc.dram_tensor("u_lhsT", [P, P], F32, kind="ExternalInput")
    v_d = nc.dram_tensor("v", [P, 1], F32, kind="ExternalInput")
    gwt_d = nc.dram_tensor("g_wT", [P, INTER], F32, kind="ExternalInput")
    owt_d = nc.dram_tensor("out_wT", [INTER, P], F32, kind="ExternalInput")
    obe_d = nc.dram_tensor("out_b_eff", [P, 1], F32, kind="ExternalInput")
    out_d = nc.dram_tensor("out", [P, N], F32, kind="ExternalOutput")

    EXP = mybir.ActivationFunctionType.Exp
    MULT = mybir.AluOpType.mult
    ADD = mybir.AluOpType.add

    with tile.TileContext(nc) as tc:
        const = tc.alloc_tile_pool(name="const", bufs=1)
        big = tc.alloc_tile_pool(name="big", bufs=1)
        probs_p = tc.alloc_tile_pool(name="probs", bufs=3)
        ysb_p = tc.alloc_tile_pool(name="ysb", bufs=2)
        rs_p = tc.alloc_tile_pool(name="rs", bufs=2)
        inv_p = tc.alloc_tile_pool(name="inv", bufs=2)
        osb_p = tc.alloc_tile_pool(name="osb", bufs=3)

        pre_ps = tc.alloc_tile_pool(name="pre_ps", bufs=2, space="PSUM")

        # ---- constants / weights ----
        ulhs = const.tile([P, P], F32, tag='ulhs')
        nc.sync.dma_start(ulhs[:], ulhs_d.ap())
        v_sb = const.tile([P, 1], F32, tag='v')
        nc.sync.dma_start(v_sb[:], v_d.ap())
        gwt = const.tile([P, INTER], F32, tag='gwt')
        nc.sync.dma_start(gwt[:], gwt_d.ap())
        owt = const.tile([INTER, P], F32, tag='owt')
        nc.sync.dma_start(owt[:], owt_d.ap())
        obe = const.tile([P, 1], F32, tag='obe')
        nc.sync.dma_start(obe[:], obe_d.ap())
        ones = const.tile([1, P], F32, tag='ones')
        nc.vector.memset(ones[:], 1.0)
        ulhs_r = const.tile([P, P], F32R, tag='ulhsr')
        nc.vector.tensor_copy(ulhs_r[:], ulhs[:])
        gwt_r = const.tile([P, INTER], F32R, tag='gwtr')
        nc.vector.tensor_copy(gwt_r[:], gwt[:])
        owt_r = const.tile([INTER, P], F32R, tag='owtr')
        nc.vector.tensor_copy(owt_r[:], owt[:])

        # ---- x (chunked DMA so compute can start early) ----
        x_sb = big.tile([P, N], F32, tag='x')
        for c in range(N // NSUB):
            nc.sync.dma_start(x_sb[:, c * NSUB:(c + 1) * NSUB],
                              x_d.ap()[:, c * NSUB:(c + 1) * NSUB])

        # ---- x_r: fp32r-rounded copy of x for PE operands ----
        x_r = big.tile([P, N], F32R, tag='xr')
        for c in range(N // NSUB):
            nc.vector.tensor_copy(x_r[:, c * NSUB:(c + 1) * NSUB],
                                  x_sb[:, c * NSUB:(c + 1) * NSUB])

        # ---- u = B x + v  (u[c1, n]) ----
        u_sb = big.tile([P, N], F32R, tag='u')
        for c in range(N // NSUB):
            u_pt = pre_ps.tile([P, NSUB], F32, tag="pre")
            nc.tensor.matmul(u_pt[:], ulhs_r[:],
                             x_r[:, c * NSUB:(c + 1) * NSUB],
                             start=True, stop=True)
            nc.vector.tensor_scalar_add(u_sb[:, c * NSUB:(c + 1) * NSUB],
                                        u_pt[:], v_sb[:])

        # ---- g_aug[m, 65*j + (0:64)] = (x^T g_w^T) block j; col 65*j+64 = 1 ----
        g_aug = big.tile([P, MBLK * (INTER + 1)], F32R, tag='gaug')
        ones_col = const.tile([P, 1], F32, tag='ones_col')
        nc.vector.memset(ones_col[:], 1.0)
        g_ones_view = g_aug.rearrange("p (j t) -> p j t", t=INTER + 1)[:, :, INTER:INTER + 1]
        nc.vector.tensor_copy(g_ones_view, ones_col[:].to_broadcast([P, MBLK, 1]))
        for j in range(MBLK):
            g_pt = pre_ps.tile([P, INTER], F32, tag="pre")
            nc.tensor.matmul(g_pt[:], x_r[:, j * P:(j + 1) * P], gwt_r[:],
                             start=True, stop=True)
            nc.vector.tensor_copy(g_aug[:, j * 65:j * 65 + INTER], g_pt[:])

        # ---- xpb = x + out_b_eff (residual + folded bias) ----
        xpb = big.tile([P, N], F32, tag='xpb')
        nc.vector.tensor_scalar_add(xpb[:], x_sb[:], obe[:])

        pre_ps.release()
        sc_ps = tc.alloc_tile_pool(name="sc_ps", bufs=2, space="PSUM")
        y_ps = tc.alloc_tile_pool(name="y_ps", bufs=1, space="PSUM")
        o_ps = tc.alloc_tile_pool(name="o_ps", bufs=1, space="PSUM")
        bc_ps = tc.alloc_tile_pool(name="bc_ps", bufs=1, space="PSUM")

        # ---- main loop over n-chunks ----
        for c in range(N // NCH):
            n0 = c * NCH
            y_t = y_ps.tile([INTER + 1, NCH], F32)
            for j in range(MBLK):
                s_t = sc_ps.tile([P, NCH], F32)
                for h in range(NCH // NSUB):
                    nc.tensor.matmul(
                        s_t[:, h * NSUB:(h + 1) * NSUB],
                        x_r[:, j * P:(j + 1) * P],
                        u_sb[:, n0 + h * NSUB:n0 + (h + 1) * NSUB],
                        start=True, stop=True)
                p_t = probs_p.tile([P, NCH], F32R)
                nc.scalar.activation(p_t[:], s_t[:], EXP)
                for h in range(NCH // NSUB):
                    nc.tensor.matmul(
                        y_t[:, h * NSUB:(h + 1) * NSUB],
                        g_aug[:, j * 65:(j + 1) * 65],
                        p_t[:, h * NSUB:(h + 1) * NSUB],
                        start=(j == 0), stop=(j == MBLK - 1))
            y_sb = ysb_p.tile([INTER, NCH], F32R)
            nc.vector.tensor_copy(y_sb[:], y_t[0:INTER, :])
            rs_sb = rs_p.tile([1, NCH], F32)
            nc.vector.tensor_copy(rs_sb[:], y_t[INTER:INTER + 1, :])
            for h in range(NCH // NSUB):
                sl = slice(h * NSUB, (h + 1) * NSUB)
                bc_t = bc_ps.tile([P, NSUB], F32)
                nc.tensor.matmul(bc_t[:], ones[:], rs_sb[:, sl],
                                 start=True, stop=True)
                inv_sb = inv_p.tile([P, NSUB], F32)
                nc.vector.reciprocal(inv_sb[:], bc_t[:])
                ot = o_ps.tile([P, NSUB], F32)
                nc.tensor.matmul(ot[:], owt_r[:], y_sb[:, sl],
                                 start=True, stop=True)
                t_sb = osb_p.tile([P, NSUB], F32)
                nc.vector.tensor_tensor(t_sb[:], ot[:], inv_sb[:], op=MULT)
                nc.vector.tensor_tensor(
                    t_sb[:], t_sb[:],
                    xpb[:, n0 + h * NSUB:n0 + (h + 1) * NSUB], op=ADD)
                nc.sync.dma_start(out_d.ap()[:, n0 + h * NSUB:n0 + (h + 1) * NSUB],
                                  t_sb[:])

        for p in (bc_ps, o_ps, y_ps, sc_ps,
                  osb_p, inv_p, rs_p, ysb_p, probs_p, big, const):
            p.release()

    nc.compile()
    return nc


def _prep_inputs(x, theta_w, theta_b, phi_w, phi_b, g_w, g_b, out_w, out_b):
    f = np.float32
    x = np.asarray(x, f)
    theta_w = np.asarray(theta_w, f)
    theta_b = np.asarray(theta_b, f)
    phi_w = np.asarray(phi_w, f)
    phi_b = np.asarray(phi_b, f)
    g_w = np.asarray(g_w, f)
    g_b = np.asarray(g_b, f)
    out_w = np.asarray(out_w, f)
    out_b = np.asarray(out_b, f)

    u_lhsT = np.ascontiguousarray(theta_w.T @ phi_w)          # [c2, c1] = B^T
    v = np.ascontiguousarray((phi_w.T @ theta_b)[:, None])    # [128, 1]
    g_wT = np.ascontiguousarray(g_w.T)                        # [128, 64]
    out_wT = np.ascontiguousarray(out_w.T)                    # [64, 128]
    out_b_eff = np.ascontiguousarray((out_w @ g_b + out_b)[:, None])

    in_maps = []
    for b in range(B):
        in_maps.append({
            "x": np.ascontiguousarray(x[b].reshape(P, N)),
            "u_lhsT": u_lhsT,
            "v": v,
            "g_wT": g_wT,
            "out_wT": out_wT,
            "out_b_eff": out_b_eff,
        })
    return in_maps


def run_on_device(inputs, trace=False, trace_cores=None):
    if "nc" not in _cached:
        _cached["nc"] = build_nc()
    nc = _cached["nc"]
    in_maps = _prep_inputs(**inputs)
    res = bass_utils.run_bass_kernel_spmd(
        nc, in_maps, core_ids=list(range(B)), trace=trace,
        trace_cores=trace_cores)
    out = np.stack([res.results[b]["out"] for b in range(B)], axis=0)
    return out.reshape(B, C, H, W).astype(np.float32), res


def kernel(**inputs):
    out, _ = run_on_device(inputs, trace=False)
    return out


# revision 8
# speedup vs baseline: 1.1417x; 1.1417x over previous
"""Non-local block (NLB) Trainium2 kernel.

Data-parallel over batch: 8 samples -> 8 NeuronCores, one sample per core.
Per core (C=128 channels, n=4096 pixels, inter=64):

  scores_T[m, n] = x_m^T (B x_n + v)        B = phi_w^T theta_w, v = phi_w^T theta_b
      (the per-n constant term theta_b . (phi_w x_n + phi_b) is dropped --
       softmax over m is invariant to it)
  probs_T = exp(scores_T)                   (scores bounded ~ +-22, fp32-safe,
                                             so no max-subtraction pass)
  y_aug[o, n]  = sum_m g_aug[m, o] probs_T[m, n]   with g_aug[:, 64] == 1
      -> row 64 is the softmax row-sum; g_aug[:, 0:64] = x^T g_w^T
  out[c, n] = (out_w @ y_aug[0:64]) [c, n] / y_aug[64, n] + out_b_eff[c] + x[c, n]
      out_b_eff = out_w @ g_b + out_b       (softmax rows sum to 1 => g_b folds)

Layouts keep the softmax (m) axis on partitions so exp runs straight out of
PSUM on ScalarE while the PE does scores / y matmuls in fp32r.
"""

import sys

if "/root/.axon_site/_ro/trn_rl_repo" not in sys.path:
    sys.path.insert(0, "/root/.axon_site/_ro/trn_rl_repo")

import types

import numpy as np

import concourse.bass as bass
import concourse.mybir as mybir
import concourse.tile as tile
from concourse import bacc
from concourse import bass_utils

# The image's antenv package lacks axon_hooks; shim it so trace=True works.
try:
    import antenv.axon_hooks  # noqa: F401
except ImportError:
    try:
        import trn_agent_boot.trn_boot as _tb

        _hook = _tb._ntff_profile_via_ctypes("/opt/axon/libaxon_pjrt.so")
        _m = types.ModuleType("antenv.axon_hooks")
        _m.get_axon_ntff_profile_hook = lambda: _hook
        sys.modules["antenv.axon_hooks"] = _m
    except Exception:
        pass

B, C, H, W = 8, 128, 64, 64
N = H * W          # 4096 pixels
INTER = C // 2     # 64
P = 128
NCH = 1024         # n-chunk width (exp batching; 2 PSUM banks)
NSUB = 512         # matmul moving-dim width (1 PSUM bank)
MBLK = N // P      # 32 m-blocks
F32 = mybir.dt.float32
F32R = mybir.dt.float32r
BF16 = mybir.dt.bfloat16

_cached = {}


def _r(ap):
    """View an fp32 AP as float32r for full-rate PE streaming."""
    return ap.bitcast(F32R)


def build_nc():
    nc = bacc.Bacc("TRN2", target_bir_lowering=False, debug=False, num_devices=B)

    x_d = nc.dram_tensor("x", [P, N], F32, kind="ExternalInput")
    ulhs_d = nc.dram_tensor("u_lhsT", [P, P], F32, kind="ExternalInput")
    v_d = nc.dram_tensor("v", [P, 1], F32, kind="ExternalInput")
    gwt_d = nc.dram_tensor("g_wT", [P, INTER], F32, kind="ExternalInput")
    owt_d = nc.dram_tensor("out_wT", [INTER, P], F32, kind="ExternalInput")
    obe_d = nc.dram_tensor("out_b_eff", [P, 1], F32, kind="ExternalInput")
    out_d = nc.dram_tensor("out", [P, N], F32, kind="ExternalOutput")

    EXP = mybir.ActivationFunctionType.Exp
    MULT = mybir.AluOpType.mult
    ADD = mybir.AluOpType.add

    with tile.TileContext(nc) as tc:
        const = tc.alloc_tile_pool(name="const", bufs=1)
        big = tc.alloc_tile_pool(name="big", bufs=1)
        probs_p = tc.alloc_tile_pool(name="probs", bufs=3)
        ysb_p = tc.alloc_tile_pool(name="ysb", bufs=2)
        rs_p = tc.alloc_tile_pool(name="rs", bufs=2)
        inv_p = tc.alloc_tile_pool(name="inv", bufs=2)
        osb_p = tc.alloc_tile_pool(name="osb", bufs=3)

        pre_ps = tc.alloc_tile_pool(name="pre_ps", bufs=2, space="PSUM")

        # ---- constants / weights ----
        ulhs = const.tile([P, P], F32, tag='ulhs')
        nc.sync.dma_start(ulhs[:], ulhs_d.ap())
        v_sb = const.tile([P, 1], F32, tag='v')
        nc.sync.dma_start(v_sb[:], v_d.ap())
        gwt = const.tile([P, INTER], F32, tag='gwt')
        nc.sync.dma_start(gwt[:], gwt_d.ap())
        owt = const.tile([INTER, P], F32, tag='owt')
        nc.sync.dma_start(owt[:], owt_d.ap())
        obe = const.tile([P, 1], F32, tag='obe')
        nc.sync.dma_start(obe[:], obe_d.ap())
        ones = const.tile([1, P], F32, tag='ones')
        nc.vector.memset(ones[:], 1.0)
        ulhs_r = const.tile([P, P], BF16, tag='ulhsr')
        nc.vector.tensor_copy(ulhs_r[:], ulhs[:])
        gwt_r = const.tile([P, INTER], BF16, tag='gwtr')
        nc.vector.tensor_copy(gwt_r[:], gwt[:])
        owt_r = const.tile([INTER, P], BF16, tag='owtr')
        nc.vector.tensor_copy(owt_r[:], owt[:])

        # ---- x (chunked DMA so compute can start early) ----
        x_sb = big.tile([P, N], F32, tag='x')
        for c in range(N // NSUB):
            nc.sync.dma_start(x_sb[:, c * NSUB:(c + 1) * NSUB],
                              x_d.ap()[:, c * NSUB:(c + 1) * NSUB])

        # ---- x_r: fp32r-rounded copy of x for PE operands ----
        x_r = big.tile([P, N], BF16, tag='xr')
        for c in range(N // NSUB):
            nc.vector.tensor_copy(x_r[:, c * NSUB:(c + 1) * NSUB],
                                  x_sb[:, c * NSUB:(c + 1) * NSUB])

        # ---- u = B x + v  (u[c1, n]) ----
        u_sb = big.tile([P, N], BF16, tag='u')
        for c in range(N // NSUB):
            u_pt = pre_ps.tile([P, NSUB], F32, tag="pre")
            nc.tensor.matmul(u_pt[:], ulhs_r[:],
                             x_r[:, c * NSUB:(c + 1) * NSUB],
                             start=True, stop=True)
            nc.vector.tensor_scalar_add(u_sb[:, c * NSUB:(c + 1) * NSUB],
                                        u_pt[:], v_sb[:])

        # ---- g_aug[m, 65*j + (0:64)] = (x^T g_w^T) block j; col 65*j+64 = 1 ----
        g_aug = big.tile([P, MBLK * (INTER + 1)], BF16, tag='gaug')
        ones_col = const.tile([P, 1], F32, tag='ones_col')
        nc.vector.memset(ones_col[:], 1.0)
        g_ones_view = g_aug.rearrange("p (j t) -> p j t", t=INTER + 1)[:, :, INTER:INTER + 1]
        nc.vector.tensor_copy(g_ones_view, ones_col[:].to_broadcast([P, MBLK, 1]))
        for j in range(MBLK):
            g_pt = pre_ps.tile([P, INTER], F32, tag="pre")
            nc.tensor.matmul(g_pt[:], x_r[:, j * P:(j + 1) * P], gwt_r[:],
                             start=True, stop=True)
            nc.vector.tensor_copy(g_aug[:, j * 65:j * 65 + INTER], g_pt[:])

        # ---- xpb = x + out_b_eff (residual + folded bias) ----
        xpb = big.tile([P, N], F32, tag='xpb')
        nc.vector.tensor_scalar_add(xpb[:], x_sb[:], obe[:])

        pre_ps.release()
        sc_ps = tc.alloc_tile_pool(name="sc_ps", bufs=2, space="PSUM")
        y_ps = tc.alloc_tile_pool(name="y_ps", bufs=1, space="PSUM")
        o_ps = tc.alloc_tile_pool(name="o_ps", bufs=1, space="PSUM")
        bc_ps = tc.alloc_tile_pool(name="bc_ps", bufs=1, space="PSUM")

        # ---- main loop over n-chunks ----
        for c in range(N // NCH):
            n0 = c * NCH
            y_t = y_ps.tile([INTER + 1, NCH], F32)
            for j in range(MBLK):
                s_t = sc_ps.tile([P, NCH], F32)
                for h in range(NCH // NSUB):
                    nc.tensor.matmul(
                        s_t[:, h * NSUB:(h + 1) * NSUB],
                        x_r[:, j * P:(j + 1) * P],
                        u_sb[:, n0 + h * NSUB:n0 + (h + 1) * NSUB],
                        start=True, stop=True)
                p_t = probs_p.tile([P, NCH], BF16)
                nc.scalar.activation(p_t[:], s_t[:], EXP)
                for h in range(NCH // NSUB):
                    nc.tensor.matmul(
                        y_t[:, h * NSUB:(h + 1) * NSUB],
                        g_aug[:, j * 65:(j + 1) * 65],
                        p_t[:, h * NSUB:(h + 1) * NSUB],
                        start=(j == 0), stop=(j == MBLK - 1))
            y_sb = ysb_p.tile([INTER, NCH], BF16)
            nc.vector.tensor_copy(y_sb[:], y_t[0:INTER, :])
            rs_sb = rs_p.tile([1, NCH], F32)
            nc.vector.tensor_copy(rs_sb[:], y_t[INTER:INTER + 1, :])
            for h in range(NCH // NSUB):
                sl = slice(h * NSUB, (h + 1) * NSUB)
                bc_t = bc_ps.tile([P, NSUB], F32)
                nc.tensor.matmul(bc_t[:], ones[:], rs_sb[:, sl],
                                 start=True, stop=True)
                inv_sb = inv_p.tile([P, NSUB], F32)
                nc.vector.reciprocal_approx_fast(out=inv_sb[:], in_=bc_t[:])
                ot = o_ps.tile([P, NSUB], F32)
                nc.tensor.matmul(ot[:], owt_r[:], y_sb[:, sl],
                                 start=True, stop=True)
                t_sb = osb_p.tile([P, NSUB], F32)
                nc.vector.tensor_tensor(t_sb[:], ot[:], inv_sb[:], op=MULT)
                nc.vector.tensor_tensor(
                    t_sb[:], t_sb[:],
                    xpb[:, n0 + h * NSUB:n0 + (h + 1) * NSUB], op=ADD)
                nc.sync.dma_start(out_d.ap()[:, n0 + h * NSUB:n0 + (h + 1) * NSUB],
                                  t_sb[:])

        for p in (bc_ps, o_ps, y_ps, sc_ps,
                  osb_p, inv_p, rs_p, ysb_p, probs_p, big, const):
            p.release()

    nc.compile()
    return nc


def _prep_inputs(x, theta_w, theta_b, phi_w, phi_b, g_w, g_b, out_w, out_b):
    f = np.float32
    x = np.asarray(x, f)
    theta_w = np.asarray(theta_w, f)
    theta_b = np.asarray(theta_b, f)
    phi_w = np.asarray(phi_w, f)
    phi_b = np.asarray(phi_b, f)
    g_w = np.asarray(g_w, f)
    g_b = np.asarray(g_b, f)
    out_w = np.asarray(out_w, f)
    out_b = np.asarray(out_b, f)

    u_lhsT = np.ascontiguousarray(theta_w.T @ phi_w)          # [c2, c1] = B^T
    v = np.ascontiguousarray((phi_w.T @ theta_b)[:, None])    # [128, 1]
    g_wT = np.ascontiguousarray(g_w.T)                        # [128, 64]
    out_wT = np.ascontiguousarray(out_w.T)                    # [64, 128]
    out_b_eff = np.ascontiguousarray((out_w @ g_b + out_b)[:, None])

    in_maps = []
    for b in range(B):
        in_maps.append({
            "x": np.ascontiguousarray(x[b].reshape(P, N)),
            "u_lhsT": u_lhsT,
            "v": v,
            "g_wT": g_wT,
            "out_wT": out_wT,
            "out_b_eff": out_b_eff,
        })
    return in_maps


def run_on_device(inputs, trace=False, trace_cores=None):
    if "nc" not in _cached:
        _cached["nc"] = build_nc()
    nc = _cached["nc"]
    in_maps = _prep_inputs(**inputs)
    res = bass_utils.run_bass_kernel_spmd(
        nc, in_maps, core_ids=list(range(B)), trace=trace,
        trace_cores=trace_cores)
    out = np.stack([res.results[b]["out"] for b in range(B)], axis=0)
    return out.reshape(B, C, H, W).astype(np.float32), res


def kernel(**inputs):
    out, _ = run_on_device(inputs, trace=False)
    return out


# revision 9
# speedup vs baseline: 1.2694x; 1.1119x over previous
"""Non-local block (NLB) Trainium2 kernel.

Data-parallel over batch: 8 samples -> 8 NeuronCores, one sample per core.
Per core (C=128 channels, n=4096 pixels, inter=64):

  scores_T[m, n] = x_m^T (B x_n + v)        B = phi_w^T theta_w, v = phi_w^T theta_b
      (the per-n constant term theta_b . (phi_w x_n + phi_b) is dropped --
       softmax over m is invariant to it)
  probs_T = exp(scores_T)                   (scores bounded ~ +-22, fp32-safe,
                                             so no max-subtraction pass)
  y_aug[o, n]  = sum_m g_aug[m, o] probs_T[m, n]   with g_aug[:, 64] == 1
      -> row 64 is the softmax row-sum; g_aug[:, 0:64] = x^T g_w^T
  out[c, n] = (out_w @ y_aug[0:64]) [c, n] / y_aug[64, n] + out_b_eff[c] + x[c, n]
      out_b_eff = out_w @ g_b + out_b       (softmax rows sum to 1 => g_b folds)

Layouts keep the softmax (m) axis on partitions so exp runs straight out of
PSUM on ScalarE while the PE does scores / y matmuls in fp32r.
"""

import sys

if "/root/.axon_site/_ro/trn_rl_repo" not in sys.path:
    sys.path.insert(0, "/root/.axon_site/_ro/trn_rl_repo")

import types

import numpy as np

import concourse.bass as bass
import concourse.mybir as mybir
import concourse.tile as tile
from concourse import bacc
from concourse import bass_utils

# The image's antenv package lacks axon_hooks; shim it so trace=True works.
try:
    import antenv.axon_hooks  # noqa: F401
except ImportError:
    try:
        import trn_agent_boot.trn_boot as _tb

        _hook = _tb._ntff_profile_via_ctypes("/opt/axon/libaxon_pjrt.so")
        _m = types.ModuleType("antenv.axon_hooks")
        _m.get_axon_ntff_profile_hook = lambda: _hook
        sys.modules["antenv.axon_hooks"] = _m
    except Exception:
        pass

B, C, H, W = 8, 128, 64, 64
N = H * W          # 4096 pixels
INTER = C // 2     # 64
P = 128
NCH = 1024         # n-chunk width (exp batching; 2 PSUM banks)
NSUB = 512         # matmul moving-dim width (1 PSUM bank)
MBLK = N // P      # 32 m-blocks
F32 = mybir.dt.float32
F32R = mybir.dt.float32r
BF16 = mybir.dt.bfloat16

_cached = {}


def _r(ap):
    """View an fp32 AP as float32r for full-rate PE streaming."""
    return ap.bitcast(F32R)


def build_nc():
    nc = bacc.Bacc("TRN2", target_bir_lowering=False, debug=False, num_devices=B)

    x_d = nc.dram_tensor("x", [P, N], F32, kind="ExternalInput")
    ulhs_d = nc.dram_tensor("u_lhsT", [P, P], F32, kind="ExternalInput")
    v_d = nc.dram_tensor("v", [P, 1], F32, kind="ExternalInput")
    gwt_d = nc.dram_tensor("g_wT", [P, INTER], F32, kind="ExternalInput")
    owt_d = nc.dram_tensor("out_wT", [INTER, P], F32, kind="ExternalInput")
    obe_d = nc.dram_tensor("out_b_eff", [P, 1], F32, kind="ExternalInput")
    out_d = nc.dram_tensor("out", [P, N], F32, kind="ExternalOutput")

    EXP = mybir.ActivationFunctionType.Exp
    MULT = mybir.AluOpType.mult
    ADD = mybir.AluOpType.add

    with tile.TileContext(nc) as tc:
        const = tc.alloc_tile_pool(name="const", bufs=1)
        big = tc.alloc_tile_pool(name="big", bufs=1)
        probs_p = tc.alloc_tile_pool(name="probs", bufs=3)
        ysb_p = tc.alloc_tile_pool(name="ysb", bufs=2)
        rs_p = tc.alloc_tile_pool(name="rs", bufs=2)
        inv_p = tc.alloc_tile_pool(name="inv", bufs=2)
        osb_p = tc.alloc_tile_pool(name="osb", bufs=3)

        # PSUM budget (8 banks): aux 2x1 + scores 2x2 + y 1x2 = 8
        aux_ps = tc.alloc_tile_pool(name="aux_ps", bufs=2, space="PSUM")
        sc_ps = tc.alloc_tile_pool(name="sc_ps", bufs=2, space="PSUM")
        y_ps = tc.alloc_tile_pool(name="y_ps", bufs=1, space="PSUM")

        # ---- constants / weights ----
        ulhs = const.tile([P, P], F32, tag='ulhs')
        nc.sync.dma_start(ulhs[:], ulhs_d.ap())
        v_sb = const.tile([P, 1], F32, tag='v')
        nc.sync.dma_start(v_sb[:], v_d.ap())
        gwt = const.tile([P, INTER], F32, tag='gwt')
        nc.sync.dma_start(gwt[:], gwt_d.ap())
        owt = const.tile([INTER, P], F32, tag='owt')
        nc.sync.dma_start(owt[:], owt_d.ap())
        obe = const.tile([P, 1], F32, tag='obe')
        nc.sync.dma_start(obe[:], obe_d.ap())
        ulhs_r = const.tile([P, P], BF16, tag='ulhsr')
        nc.vector.tensor_copy(ulhs_r[:], ulhs[:])
        gwt_r = const.tile([P, INTER], BF16, tag='gwtr')
        nc.vector.tensor_copy(gwt_r[:], gwt[:])
        owt_r = const.tile([INTER, P], BF16, tag='owtr')
        nc.vector.tensor_copy(owt_r[:], owt[:])
        ones_col = const.tile([P, 1], F32, tag='ones_col')
        nc.vector.memset(ones_col[:], 1.0)

        x_sb = big.tile([P, N], F32, tag='x')
        x_r = big.tile([P, N], BF16, tag='xr')
        u_sb = big.tile([P, N], BF16, tag='u')
        xpb = big.tile([P, N], F32, tag='xpb')
        g_aug = big.tile([P, MBLK * (INTER + 1)], BF16, tag='gaug')

        # ---- prologue, interleaved per 512-chunk ----
        for c in range(N // NSUB):
            sl = slice(c * NSUB, (c + 1) * NSUB)
            nc.sync.dma_start(x_sb[:, sl], x_d.ap()[:, sl])
            nc.vector.tensor_copy(x_r[:, sl], x_sb[:, sl])
            u_pt = aux_ps.tile([P, NSUB], F32, tag="aux")
            nc.tensor.matmul(u_pt[:], ulhs_r[:], x_r[:, sl],
                             start=True, stop=True)
            nc.vector.tensor_scalar_add(u_sb[:, sl], u_pt[:], v_sb[:])
            nc.vector.tensor_scalar_add(xpb[:, sl], x_sb[:, sl], obe[:])
            for j in range(c * NSUB // P, (c + 1) * NSUB // P):
                g_pt = aux_ps.tile([P, INTER], F32, tag="aux")
                nc.tensor.matmul(g_pt[:], x_r[:, j * P:(j + 1) * P], gwt_r[:],
                                 start=True, stop=True)
                nc.vector.tensor_copy(g_aug[:, j * 65:j * 65 + INTER], g_pt[:])
        g_ones_view = g_aug.rearrange("p (j t) -> p j t", t=INTER + 1)[:, :, INTER:INTER + 1]
        nc.vector.tensor_copy(g_ones_view, ones_col[:].to_broadcast([P, MBLK, 1]))

        # ---- main loop over n-chunks ----
        for c in range(N // NCH):
            n0 = c * NCH
            y_t = y_ps.tile([INTER + 1, NCH], F32)
            for j in range(MBLK):
                s_t = sc_ps.tile([P, NCH], F32)
                for h in range(NCH // NSUB):
                    nc.tensor.matmul(
                        s_t[:, h * NSUB:(h + 1) * NSUB],
                        x_r[:, j * P:(j + 1) * P],
                        u_sb[:, n0 + h * NSUB:n0 + (h + 1) * NSUB],
                        start=True, stop=True)
                p_t = probs_p.tile([P, NCH], BF16)
                nc.scalar.activation(p_t[:], s_t[:], EXP)
                for h in range(NCH // NSUB):
                    nc.tensor.matmul(
                        y_t[:, h * NSUB:(h + 1) * NSUB],
                        g_aug[:, j * 65:(j + 1) * 65],
                        p_t[:, h * NSUB:(h + 1) * NSUB],
                        start=(j == 0), stop=(j == MBLK - 1))
            y_sb = ysb_p.tile([INTER, NCH], BF16)
            nc.vector.tensor_copy(y_sb[:], y_t[0:INTER, :])
            rs_sb = rs_p.tile([1, NCH], F32)
            nc.vector.tensor_copy(rs_sb[:], y_t[INTER:INTER + 1, :])
            rs_bc = inv_p.tile([P, NCH], F32, tag='rsbc')
            nc.gpsimd.partition_broadcast(rs_bc[:], rs_sb[:])
            inv_sb = inv_p.tile([P, NCH], F32, tag='inv')
            nc.vector.reciprocal_approx_fast(out=inv_sb[:], in_=rs_bc[:])
            for h in range(NCH // NSUB):
                sl = slice(h * NSUB, (h + 1) * NSUB)
                ot = aux_ps.tile([P, NSUB], F32, tag="aux")
                nc.tensor.matmul(ot[:], owt_r[:], y_sb[:, sl],
                                 start=True, stop=True)
                t_sb = osb_p.tile([P, NSUB], F32)
                nc.vector.tensor_tensor(t_sb[:], ot[:], inv_sb[:, sl], op=MULT)
                nc.vector.tensor_tensor(
                    t_sb[:], t_sb[:],
                    xpb[:, n0 + h * NSUB:n0 + (h + 1) * NSUB], op=ADD)
                nc.sync.dma_start(out_d.ap()[:, n0 + h * NSUB:n0 + (h + 1) * NSUB],
                                  t_sb[:])

        for p in (y_ps, sc_ps, aux_ps,
                  osb_p, inv_p, rs_p, ysb_p, probs_p, big, const):
            p.release()

    nc.compile()
    return nc


def _prep_inputs(x, theta_w, theta_b, phi_w, phi_b, g_w, g_b, out_w, out_b):
    f = np.float32
    x = np.asarray(x, f)
    theta_w = np.asarray(theta_w, f)
    theta_b = np.asarray(theta_b, f)
    phi_w = np.asarray(phi_w, f)
    phi_b = np.asarray(phi_b, f)
    g_w = np.asarray(g_w, f)
    g_b = np.asarray(g_b, f)
    out_w = np.asarray(out_w, f)
    out_b = np.asarray(out_b, f)

    u_lhsT = np.ascontiguousarray(theta_w.T @ phi_w)          # [c2, c1] = B^T
    v = np.ascontiguousarray((phi_w.T @ theta_b)[:, None])    # [128, 1]
    g_wT = np.ascontiguousarray(g_w.T)                        # [128, 64]
    out_wT = np.ascontiguousarray(out_w.T)                    # [64, 128]
    out_b_eff = np.ascontiguousarray((out_w @ g_b + out_b)[:, None])

    in_maps = []
    for b in range(B):
        in_maps.append({
            "x": np.ascontiguousarray(x[b].reshape(P, N)),
            "u_lhsT": u_lhsT,
            "v": v,
            "g_wT": g_wT,
            "out_wT": out_wT,
            "out_b_eff": out_b_eff,
        })
    return in_maps


def run_on_device(inputs, trace=False, trace_cores=None):
    if "nc" not in _cached:
        _cached["nc"] = build_nc()
    nc = _cached["nc"]
    in_maps = _prep_inputs(**inputs)
    res = bass_utils.run_bass_kernel_spmd(
        nc, in_maps, core_ids=list(range(B)), trace=trace,
        trace_cores=trace_cores)
    out = np.stack([res.results[b]["out"] for b in range(B)], axis=0)
    return out.reshape(B, C, H, W).astype(np.float32), res


def kernel(**inputs):
    out, _ = run_on_device(inputs, trace=False)
    return out
